# revision 7
# baseline (speedup 1.0000x reference)
import sys
sys.path.insert(0, '/opt/trn_rl_repo')
import numpy as np

DIM = 1024
H = 16
HD = 64
T = 2048
NCORES = 8
HPC = H // NCORES          # heads per core = 2
DL = HPC * HD              # local dims per core = 128
NT = T // 128              # 16 t-tiles

_cache = {"nc": None}


def _softplus(x):
    return np.log1p(np.exp(-abs(x))) + max(x, 0.0)


def _rotary_tables():
    # mimic reference's f32 computation (jax on cpu if available)
    try:
        import jax
        import jax.numpy as jnp
        with jax.default_device(jax.devices("cpu")[0]):
            nf = HD // 4
            af = (1.0 / 1024.0) ** jnp.linspace(0.0, 1.0, nf, dtype=jnp.float32)
            af = jnp.concatenate([af, jnp.zeros(nf, dtype=jnp.float32)])
            t = jnp.arange(T, dtype=jnp.float32)
            theta = t[:, None] * af[None, :]
            return np.asarray(jnp.cos(theta)), np.asarray(jnp.sin(theta))
    except Exception:
        nf = HD // 4
        af = (np.float32(1.0 / 1024.0) ** np.linspace(0.0, 1.0, nf, dtype=np.float32)).astype(np.float32)
        af = np.concatenate([af, np.zeros(nf, np.float32)])
        theta = np.arange(T, dtype=np.float32)[:, None] * af[None, :]
        return np.cos(theta).astype(np.float32), np.sin(theta).astype(np.float32)


def _build_nc():
    import concourse.bass as bass
    from concourse import bacc, mybir
    import concourse.tile as tile

    F32 = mybir.dt.float32
    F32R = mybir.dt.float32r
    BF16 = mybir.dt.bfloat16
    AF = mybir.ActivationFunctionType

    nc = bacc.Bacc("TRN2", target_bir_lowering=False, debug=False)
    # all per-core payload packed into ONE bf16 tensor (each external tensor
    # costs ~12ms of per-call transfer overhead); identity/causal-mask/ones
    # are generated on-chip, the two f32 norm scales ride as raw bytes
    # big: xs 0:2048 | ves 2048:4096 | WTa 4096:7168 | WpT 7168:8192 |
    #      cos 8192:8704 | sin 8704:9216 | scl(f32 bytes) 9216:9220
    d_big = nc.dram_tensor("big", [128, 9220], BF16, kind="ExternalInput")
    d_out = nc.dram_tensor("out", [T // NCORES, DIM], BF16, kind="ExternalOutput")
    d_xs = d_big[:, 0:2048]
    d_ves = d_big[:, 2048:4096].rearrange("p (t d) -> p t d", d=128)
    d_WTa = d_big[:, 4096:7168].rearrange("p (k j) -> p k j", j=3 * DL)
    d_WpT = d_big[:, 7168:8192]
    d_cos = d_big[:, 8192:8704].rearrange("p (t f) -> p t f", f=32)
    d_sin = d_big[:, 8704:9216].rearrange("p (t f) -> p t f", f=32)
    d_scl = d_big[:, 9216:9220].bitcast(F32)
    I32 = mybir.dt.int32

    CW = 386  # per-tile col layout: q 0:128 | k 128:256 | vh0 256:320 | 1s 320 | vh1 321:385 | 1s 385
    RG = [list(range(NCORES))]

    with tile.TileContext(nc) as tc:
        with tc.tile_pool(name="persist", bufs=1) as P, \
             tc.tile_pool(name="dram", bufs=1, space="DRAM") as D:
            xTa = P.tile([128, 8, T], BF16, tag="xTa")
            ves = P.tile([128, NT, 128], F32, tag="ves")
            qkv = P.tile([128, NT, CW], F32R, tag="qkv")
            cos4 = P.tile([128, NT, 4, 32], F32, tag="cos4")
            sin4 = P.tile([128, NT, 4, 32], F32, tag="sin4")
            qrT = P.tile([128, T], F32R, tag="qrT")
            krT = P.tile([128, T], F32R, tag="krT")
            yT = P.tile([128, T], BF16, tag="yT")
            WpT = P.tile([128, DIM], BF16, tag="WpT")
            idn = P.tile([128, 128], F32R, tag="idn")
            msk = P.tile([128, 128], F32, tag="msk")
            on1 = P.tile([1, 64], F32R, tag="on1")
            scl = P.tile([128, 2], F32, tag="scl")
            rd = P.tile([1, 2 * T], F32R, tag="rd")  # recip denominators, head h at cols [h*T, (h+1)*T)
            rdf = P.tile([1, 2 * T], F32, tag="rdf")

            ag_in = D.tile([128, T], BF16, tag="ag_in")
            ag_out = D.tile([8, 128, T], BF16, tag="ag_out")
            rs_in = D.tile([T, DIM], F32, tag="rs_in")
            rs_out = D.tile([T // NCORES, DIM], F32, tag="rs_out")

            # ---- allgather x across the 8 cores (each uploads 1/8 of xT) ----
            nc.gpsimd.dma_start(out=ag_in[:, :], in_=d_xs[:, :])
            nc.gpsimd.collective_compute(
                "AllGather", mybir.AluOpType.bypass, replica_groups=RG,
                ins=[ag_in[:, :].opt()], outs=[ag_out[:, :, :].opt()])
            for k in range(8):
                nc.sync.dma_start(out=xTa[:, k, :], in_=ag_out[k, :, :])

            # ---- small loads + on-chip constants ----
            nc.sync.dma_start(out=WpT, in_=d_WpT[:, :])
            nc.sync.dma_start(out=scl, in_=d_scl)
            # identity / causal mask / ones generated on-chip from an iota
            it = P.tile([128, 128], I32, tag="it")
            nc.gpsimd.iota(it[:, :], pattern=[[1, 128]], base=0, channel_multiplier=-1)
            nc.gpsimd.tensor_scalar(idn[:, :], it[:, :], 0, None, op0=mybir.AluOpType.is_equal)
            nc.gpsimd.tensor_scalar(msk[:, :], it[:, :], 0, None, op0=mybir.AluOpType.is_ge)
            nc.gpsimd.tensor_scalar(on1[:, :], it[0:1, 0:64], -1 << 20, None, op0=mybir.AluOpType.is_ge)
            one16 = it[:, 0:16].rearrange("p (t o) -> p t o", o=1)
            nc.gpsimd.tensor_scalar(qkv[:, :, 320:321], one16, -1 << 20, None, op0=mybir.AluOpType.is_ge)
            nc.gpsimd.tensor_scalar(qkv[:, :, 385:386], one16, -1 << 20, None, op0=mybir.AluOpType.is_ge)

            with tc.tile_pool(name="phaseA", bufs=1) as A, \
                 tc.tile_pool(name="grp", bufs=2) as G, \
                 tc.tile_pool(name="qkvps", bufs=3, space="PSUM") as QPS, \
                 tc.tile_pool(name="tps", bufs=2, space="PSUM") as TPS:
                WTa = A.tile([128, 8, 3 * DL], BF16, tag="WTa")
                nc.sync.dma_start(out=WTa, in_=d_WTa[:, :, :])
                vesb = A.tile([128, NT, 128], BF16, tag="vesb")
                nc.sync.dma_start(out=vesb, in_=d_ves[:, :, :])
                nc.vector.tensor_copy(ves[:, :, :], vesb[:, :, :])
                cosc = A.tile([128, NT, 32], BF16, tag="cosc")
                sinc = A.tile([128, NT, 32], BF16, tag="sinc")
                nc.sync.dma_start(out=cosc, in_=d_cos[:, :, :])
                nc.sync.dma_start(out=sinc, in_=d_sin[:, :, :])
                for a in range(4):
                    nc.vector.tensor_copy(cos4[:, :, a, :], cosc[:, :, :])
                    nc.vector.tensor_copy(sin4[:, :, a, :], sinc[:, :, :])

                for g in range(4):
                    for ii in range(4):
                        i = 4 * g + ii
                        ps = QPS.tile([128, 3 * DL], F32, tag="qkvps")
                        for k in range(8):
                            nc.tensor.matmul(ps[:, :], xTa[:, k, 128 * i:128 * (i + 1)],
                                             WTa[:, k, :], start=(k == 0), stop=(k == 7))
                        nc.scalar.copy(qkv[:, i, 0:256], ps[:, 0:256])
                        # v with residual: psum cols 256:320 -> 256:320 ; 320:384 -> 321:385
                        nc.vector.tensor_add(qkv[:, i, 256:320], ps[:, 256:320], ves[:, i, 0:64])
                        nc.vector.tensor_add(qkv[:, i, 321:385], ps[:, 320:384], ves[:, i, 64:128])
                    # ---- norm + rotary for group g (tiles 4g..4g+3) ----
                    sqg = G.tile([128, 4, 256], F32, tag="sqg")
                    for ii in range(4):
                        i = 4 * g + ii
                        nc.scalar.activation(sqg[:, ii, :], qkv[:, i, 0:256].bitcast(F32), AF.Square)
                    # red layout: [128, group4, tile4] so q-groups (0:2) and k-groups (2:4) are contiguous
                    red = G.tile([128, 4, 4], F32, tag="red")
                    nc.vector.tensor_reduce(red[:, :, :].transpose([0, 2, 1]),
                                            sqg[:, :, :].rearrange("p t (a d) -> p t a d", d=64),
                                            axis=mybir.AxisListType.X, op=mybir.AluOpType.add)
                    rno = G.tile([128, 4, 4], F32, tag="rno")
                    nc.scalar.activation(rno[:, 0:2, :], red[:, 0:2, :], AF.Sqrt, scale=scl[:, 0:1])
                    nc.scalar.activation(rno[:, 2:4, :], red[:, 2:4, :], AF.Sqrt, scale=scl[:, 1:2])
                    rin = G.tile([128, 4, 4], F32, tag="rin")
                    nc.vector.reciprocal(rin[:, :, :], rno[:, :, :])
                    for ii in range(4):
                        i = 4 * g + ii
                        for g4 in range(4):
                            nc.vector.tensor_scalar_mul(
                                qkv[:, i, 64 * g4:64 * (g4 + 1)],
                                qkv[:, i, 64 * g4:64 * (g4 + 1)].bitcast(F32),
                                rin[:, g4, ii:ii + 1])
                    # rotary in place: x1 = cols (4g4)*64 .. +32 ; x2 = +32
                    x1 = qkv[:, 4 * g:4 * g + 4, 0:256].rearrange("p t (a d) -> p t a d", d=64)[:, :, :, 0:32]
                    x2 = qkv[:, 4 * g:4 * g + 4, 0:256].rearrange("p t (a d) -> p t a d", d=64)[:, :, :, 32:64]
                    cg = cos4[:, 4 * g:4 * g + 4, :, :]
                    sg = sin4[:, 4 * g:4 * g + 4, :, :]
                    t3 = G.tile([128, 4, 4, 32], F32, tag="t3")
                    t4 = G.tile([128, 4, 4, 32], F32, tag="t4")
                    y2s = G.tile([128, 4, 4, 32], F32, tag="y2s")
                    nc.vector.tensor_mul(t3[:, :, :, :], x1.bitcast(F32), sg)
                    nc.vector.tensor_mul(t4[:, :, :, :], x2.bitcast(F32), cg)
                    nc.vector.tensor_sub(y2s[:, :, :, :], t4[:, :, :, :], t3[:, :, :, :])
                    nc.vector.tensor_mul(t3[:, :, :, :], x1.bitcast(F32), cg)
                    nc.vector.tensor_mul(t4[:, :, :, :], x2.bitcast(F32), sg)
                    nc.vector.tensor_add(x1, t3[:, :, :, :], t4[:, :, :, :])
                    nc.vector.tensor_copy(x2, y2s[:, :, :, :])
                    # ---- transposes of q,k for group ----
                    ptq = TPS.tile([128, 512], F32R, tag="ptq")
                    ptk = TPS.tile([128, 512], F32R, tag="ptk")
                    for ii in range(4):
                        i = 4 * g + ii
                        nc.tensor.transpose(ptq[:, 128 * ii:128 * (ii + 1)], qkv[:, i, 0:128], idn[:, :])
                        nc.tensor.transpose(ptk[:, 128 * ii:128 * (ii + 1)], qkv[:, i, 128:256], idn[:, :])
                    nc.scalar.copy(qrT[:, 512 * g:512 * (g + 1)], ptq[:, :].bitcast(F32))
                    nc.scalar.copy(krT[:, 512 * g:512 * (g + 1)], ptk[:, :].bitcast(F32))

            # ================= attention =================
            with tc.tile_pool(name="sps", bufs=2, space="PSUM") as SPS, \
                 tc.tile_pool(name="yps", bufs=1, space="PSUM") as YPS, \
                 tc.tile_pool(name="eps", bufs=3) as EPS:
                for h in range(2):
                    yw = []
                    for w in range(4):
                        t_ = YPS.tile([65, 512], F32, tag=f"yw{w}")
                        yw.append(t_)
                    for j in range(NT):
                        lk = krT[64 * h:64 * (h + 1), 128 * j:128 * (j + 1)]
                        cs_al = 512 * (j // 4)
                        chunks = [(cs_al, 1024 * (cs_al // 1024 + 1))]
                        q0 = cs_al // 1024 + 1
                        while 1024 * q0 < T:
                            chunks.append((1024 * q0, 1024 * (q0 + 1)))
                            q0 += 1
                        off = 128 * (j % 4)  # diag offset within first chunk
                        for (cs, ce) in chunks:
                            wdt = ce - cs
                            psc = SPS.tile([128, 1024], F32, tag="psc")
                            for p0 in range(cs, ce, 512):
                                nc.tensor.matmul(psc[:, p0 - cs:p0 + 512 - cs], lk,
                                                 qrT[64 * h:64 * (h + 1), p0:p0 + 512],
                                                 start=True, stop=True)
                            es = EPS.tile([128, 1024], F32R, tag="es")
                            nc.scalar.activation(es[:, 0:wdt], psc[:, 0:wdt], AF.Exp)
                            if cs == cs_al:
                                if off > 0:
                                    nc.vector.tensor_scalar_mul(es[:, 0:off], es[:, 0:off].bitcast(F32), 0.0)
                                nc.vector.tensor_mul(es[:, off:off + 128], es[:, off:off + 128].bitcast(F32), msk[:, :])
                            # PV pieces (all full 512, zero-offset)
                            lv = qkv[:, j, 256 + 65 * h:256 + 65 * h + 65]
                            for p0 in range(cs, ce, 512):
                                w = p0 // 512
                                nc.tensor.matmul(yw[w][:, :], lv, es[:, p0 - cs:p0 + 512 - cs],
                                                 start=(j == 0), stop=(j == min(15, 4 * w + 3)))
                    # normalize: recip of denom rows, bcast via ones matmul, divide
                    for w in range(4):
                        c0 = h * T + 512 * w
                        nc.vector.reciprocal(rdf[0:1, c0:c0 + 512], yw[w][64:65, :])
                        nc.vector.tensor_scalar_mul(rd[0:1, c0:c0 + 512], rdf[0:1, c0:c0 + 512], 1.0)
                        pb = SPS.tile([64, 512], F32, tag="psc")
                        nc.tensor.matmul(pb[:, :], on1[:, :], rd[0:1, c0:c0 + 512], start=True, stop=True)
                        ycp = EPS.tile([64, 512], F32, tag="ycp")
                        nc.scalar.copy(ycp[:, :], yw[w][0:64, :])
                        nc.vector.tensor_mul(yT[64 * h:64 * (h + 1), 512 * w:512 * (w + 1)],
                                             ycp[:, :], pb[:, :])

            # ================= output projection + reduce-scatter =================
            with tc.tile_pool(name="ops", bufs=3, space="PSUM") as OPS, \
                 tc.tile_pool(name="ost", bufs=3) as OST:
                for i in range(NT):
                    po = OPS.tile([128, 1024], F32, tag="po")
                    nc.tensor.matmul(po[:, 0:512], yT[:, 128 * i:128 * (i + 1)], WpT[:, 0:512], start=True, stop=True)
                    nc.tensor.matmul(po[:, 512:1024], yT[:, 128 * i:128 * (i + 1)], WpT[:, 512:1024], start=True, stop=True)
                    ob = OST.tile([128, 1024], F32, tag="ob")
                    if i % 2 == 0:
                        nc.scalar.copy(ob[:, :], po[:, :])
                    else:
                        nc.vector.tensor_copy(ob[:, :], po[:, :])
                    nc.sync.dma_start(out=rs_in[128 * i:128 * (i + 1), :], in_=ob[:, :])
                nc.gpsimd.collective_compute(
                    "ReduceScatter", mybir.AluOpType.add, replica_groups=RG,
                    ins=[rs_in[:, :].opt()], outs=[rs_out[:, :].opt()])
                # downcast the final 256 rows to bf16 for the host download
                of = OST.tile([128, 2, DIM], F32, tag="of")
                og = OST.tile([128, 2, DIM], BF16, tag="og")
                for r in range(2):
                    nc.sync.dma_start(out=of[:, r, :], in_=rs_out[128 * r:128 * (r + 1), :])
                nc.vector.tensor_copy(og[:, :, :], of[:, :, :])
                for r in range(2):
                    nc.sync.dma_start(out=d_out[128 * r:128 * (r + 1), :], in_=og[:, r, :])
    nc.compile()
    return nc


def _prep_inputs(x, ve, c_q, c_k, c_v, qkv_scale, q_scale, k_scale, v_lambda, c_proj, c_proj_scale):
    import ml_dtypes
    BF = ml_dtypes.bfloat16
    x = np.asarray(x, np.float32)[0]          # [T, DIM]
    ve = np.asarray(ve, np.float32)[0]
    W = np.asarray(qkv_scale, np.float32)[:, None] * np.concatenate(
        [np.asarray(c_q, np.float32), np.asarray(c_k, np.float32), np.asarray(c_v, np.float32)], axis=0)
    spq = _softplus(float(np.asarray(q_scale)))
    spk = _softplus(float(np.asarray(k_scale)))
    spv = _softplus(float(np.asarray(v_lambda)))
    cos, sin = _rotary_tables()               # [T, 32]

    xT = np.ascontiguousarray(x.T).astype(BF)  # [DIM, T]
    cosc = np.ascontiguousarray(cos.reshape(NT, 128, 32).transpose(1, 0, 2)).astype(BF)
    sinc = np.ascontiguousarray(sin.reshape(NT, 128, 32).transpose(1, 0, 2)).astype(BF)
    sclf = np.empty((128, 2), np.float32)
    sclf[:, 0] = 1.0 / (spq * spq)
    sclf[:, 1] = 1.0 / (64.0 * spk * spk)
    scl_bf_bytes = sclf.view(BF)  # [128, 4] raw f32 bytes as bf16 pairs

    Wp = np.asarray(c_proj_scale, np.float32)[None, :] * np.asarray(c_proj, np.float32)  # [e, d]
    vesf = spv * ve  # [T, DIM] f32

    in_maps = []
    for c in range(NCORES):
        r0 = DL * c
        Wc = np.concatenate([W[r0:r0 + DL], W[DIM + r0:DIM + r0 + DL], W[2 * DIM + r0:2 * DIM + r0 + DL]], axis=0)  # [384, 1024]
        WTc = np.ascontiguousarray(Wc.T)      # [1024, 384]
        big = np.empty((128, 9220), BF)
        big[:, 0:2048] = xT[128 * c:128 * (c + 1), :]
        big[:, 2048:4096] = vesf[:, r0:r0 + DL].reshape(NT, 128, 128).transpose(1, 0, 2).reshape(128, 2048).astype(BF)
        big[:, 4096:7168] = WTc.reshape(8, 128, 3 * DL).transpose(1, 0, 2).reshape(128, 3072).astype(BF)
        big[:, 7168:8192] = Wp[:, r0:r0 + DL].T.astype(BF)
        big[:, 8192:8704] = cosc.reshape(128, 512)
        big[:, 8704:9216] = sinc.reshape(128, 512)
        big[:, 9216:9220] = scl_bf_bytes
        in_maps.append({"big": big})
    return in_maps


def _enable_jax_compile_cache():
    # content-addressed persistent compile cache: the spmd runner re-traces
    # and re-lowers its jitted body on every invocation, which re-runs the
    # neuronx-cc lowering hook (~0.5s) unless the persistent cache absorbs it
    import os
    try:
        import jax
        jax.config.update("jax_compilation_cache_dir",
                          os.path.expanduser("~/.cache/jax_bass_cc"))
    except Exception:
        pass
    try:
        import jax
        jax.config.update("jax_persistent_cache_min_compile_time_secs", 0.0)
    except Exception:
        pass
    try:
        import jax
        jax.config.update("jax_persistent_cache_min_entry_size_bytes", -1)
    except Exception:
        pass


def kernel(x, ve, c_q, c_k, c_v, qkv_scale, q_scale, k_scale, v_lambda, c_proj, c_proj_scale, _trace=False):
    from concourse.bass_utils import run_bass_kernel_spmd
    _enable_jax_compile_cache()
    if _cache["nc"] is None:
        _cache["nc"] = _build_nc()
    nc = _cache["nc"]
    in_maps = _prep_inputs(x, ve, c_q, c_k, c_v, qkv_scale, q_scale, k_scale, v_lambda, c_proj, c_proj_scale)
    import time as _time
    res = run_bass_kernel_spmd(nc, in_maps, core_ids=list(range(NCORES)))
    res = run_bass_kernel_spmd(nc, in_maps, core_ids=list(range(NCORES)))
    t0 = _time.time()
    res = run_bass_kernel_spmd(nc, in_maps, core_ids=list(range(NCORES)))
    kernel.last_exec_wall_ns = int((_time.time() - t0) * 1e9)
    out = np.concatenate([r["out"] for r in res.results], axis=0)
    kernel.last_results = res
    return np.ascontiguousarray(out.astype(np.float32))[None, :, :]


# revision 12
# speedup vs baseline: 1.1162x; 1.1162x over previous
import sys
sys.path.insert(0, '/opt/trn_rl_repo')
import numpy as np

DIM = 1024
H = 16
HD = 64
T = 2048
NCORES = 8
HPC = H // NCORES          # heads per core = 2
DL = HPC * HD              # local dims per core = 128
NT = T // 128              # 16 t-tiles

_cache = {"nc": None}


def _softplus(x):
    return np.log1p(np.exp(-abs(x))) + max(x, 0.0)


def _build_nc():
    import concourse.bass as bass
    from concourse import bacc, mybir
    import concourse.tile as tile

    F32 = mybir.dt.float32
    F32R = mybir.dt.float32r
    BF16 = mybir.dt.bfloat16
    AF = mybir.ActivationFunctionType

    nc = bacc.Bacc("TRN2", target_bir_lowering=False, debug=False)
    # all per-core payload packed into ONE bf16 tensor (each external tensor
    # costs ~12ms of per-call transfer overhead); identity/causal-mask/ones
    # and the rotary tables are generated on-chip, the two f32 norm scales
    # ride as raw bytes
    # big: xs 0:2048 | ves 2048:4096 | WTa 4096:7168 | WpT 7168:8192 |
    #      scl(f32 bytes) 8192:8196
    d_big = nc.dram_tensor("big", [128, 8196], BF16, kind="ExternalInput")
    d_out = nc.dram_tensor("out", [T // NCORES, DIM], BF16, kind="ExternalOutput")
    d_xs = d_big[:, 0:2048]
    d_ves = d_big[:, 2048:4096].rearrange("p (t d) -> p t d", d=128)
    d_WTa = d_big[:, 4096:7168].rearrange("p (k j) -> p k j", j=3 * DL)
    d_WpT = d_big[:, 7168:8192]
    d_scl = d_big[:, 8192:8196].bitcast(F32)
    I32 = mybir.dt.int32
    # rotary angular frequencies (f32, exactly as the reference computes them);
    # the upper 16 frequencies are zero -> cos=1, sin=0
    af32 = ((np.float32(1.0) / np.float32(1024.0))
            ** np.linspace(0.0, 1.0, 16, dtype=np.float32)).astype(np.float32)
    PI = float(np.float32(np.pi))
    TWO_PI = float(np.float32(2.0 * np.pi))
    INV_2PI = float(np.float32(1.0 / (2.0 * np.pi)))
    PI_2 = float(np.float32(np.pi / 2.0))

    CW = 386  # per-tile col layout: q 0:128 | k 128:256 | vh0 256:320 | 1s 320 | vh1 321:385 | 1s 385
    RG = [list(range(NCORES))]

    with tile.TileContext(nc) as tc:
        with tc.tile_pool(name="persist", bufs=1) as P, \
             tc.tile_pool(name="dram", bufs=1, space="DRAM") as D:
            xTa = P.tile([128, 8, T], BF16, tag="xTa")
            ves = P.tile([128, NT, 128], F32, tag="ves")
            qkv = P.tile([128, NT, CW], F32R, tag="qkv")
            cos4 = P.tile([128, NT, 4, 32], F32, tag="cos4")
            sin4 = P.tile([128, NT, 4, 32], F32, tag="sin4")
            qrT = P.tile([128, T], F32R, tag="qrT")
            krT = P.tile([128, T], F32R, tag="krT")
            yT = P.tile([128, T], BF16, tag="yT")
            WpT = P.tile([128, DIM], BF16, tag="WpT")
            idn = P.tile([128, 128], F32R, tag="idn")
            msk = P.tile([128, 128], F32, tag="msk")
            on1 = P.tile([1, 64], F32R, tag="on1")
            scl = P.tile([128, 2], F32, tag="scl")
            rd = P.tile([1, 2 * T], F32R, tag="rd")  # recip denominators, head h at cols [h*T, (h+1)*T)
            rdf = P.tile([1, 2 * T], F32, tag="rdf")

            ag_in = D.tile([128, T], BF16, tag="ag_in")
            ag_out = D.tile([8, 128, T], BF16, tag="ag_out")
            rs_in = D.tile([T, DIM], F32, tag="rs_in")
            rs_out = D.tile([T // NCORES, DIM], F32, tag="rs_out")

            # ---- allgather x across the 8 cores (each uploads 1/8 of xT) ----
            nc.gpsimd.dma_start(out=ag_in[:, :], in_=d_xs[:, :])
            nc.gpsimd.collective_compute(
                "AllGather", mybir.AluOpType.bypass, replica_groups=RG,
                ins=[ag_in[:, :].opt()], outs=[ag_out[:, :, :].opt()])
            for k in range(8):
                nc.sync.dma_start(out=xTa[:, k, :], in_=ag_out[k, :, :])

            # ---- small loads + on-chip constants ----
            nc.sync.dma_start(out=WpT, in_=d_WpT[:, :])
            nc.sync.dma_start(out=scl, in_=d_scl)
            # identity / causal mask / ones generated on-chip from an iota
            it = P.tile([128, 128], I32, tag="it")
            nc.gpsimd.iota(it[:, :], pattern=[[1, 128]], base=0, channel_multiplier=-1)
            nc.gpsimd.tensor_scalar(idn[:, :], it[:, :], 0, None, op0=mybir.AluOpType.is_equal)
            nc.gpsimd.tensor_scalar(msk[:, :], it[:, :], 0, None, op0=mybir.AluOpType.is_ge)
            nc.gpsimd.tensor_scalar(on1[:, :], it[0:1, 0:64], -1 << 20, None, op0=mybir.AluOpType.is_ge)
            one16 = it[:, 0:16].rearrange("p (t o) -> p t o", o=1)
            nc.gpsimd.tensor_scalar(qkv[:, :, 320:321], one16, -1 << 20, None, op0=mybir.AluOpType.is_ge)
            nc.gpsimd.tensor_scalar(qkv[:, :, 385:386], one16, -1 << 20, None, op0=mybir.AluOpType.is_ge)

            with tc.tile_pool(name="phaseA", bufs=1) as A, \
                 tc.tile_pool(name="grp", bufs=2) as G, \
                 tc.tile_pool(name="qkvps", bufs=3, space="PSUM") as QPS, \
                 tc.tile_pool(name="tps", bufs=2, space="PSUM") as TPS:
                WTa = A.tile([128, 8, 3 * DL], BF16, tag="WTa")
                nc.sync.dma_start(out=WTa, in_=d_WTa[:, :, :])
                vesb = A.tile([128, NT, 128], BF16, tag="vesb")
                nc.sync.dma_start(out=vesb, in_=d_ves[:, :, :])
                nc.vector.tensor_copy(ves[:, :, :], vesb[:, :, :])
                # ---- rotary tables on-chip: theta = t*af in f32 (bit-exact vs
                # reference), range-reduce to [-pi,pi] via round-to-nearest
                # f32->i32 conversion, then the Sin activation table ----
                it2 = A.tile([128, NT], I32, tag="it2")
                nc.gpsimd.iota(it2[:, :], pattern=[[128, NT]], base=0, channel_multiplier=1)
                tf = A.tile([128, NT], F32, tag="tf")
                nc.vector.tensor_copy(tf[:, :], it2[:, :])
                tf1 = tf[:, :].rearrange("p (t o) -> p t o", o=1)
                th = A.tile([128, NT, 16], F32, tag="th")
                for f in range(16):
                    nc.vector.tensor_scalar_mul(th[:, :, f:f + 1], tf1, float(af32[f]))
                ph = A.tile([128, NT, 16], F32, tag="ph")
                tmp = A.tile([128, NT, 16], F32, tag="tmp")
                tmpi = A.tile([128, NT, 16], I32, tag="tmpi")
                for (src, dst) in ((th, sin4), (ph, cos4)):
                    if src is ph:
                        nc.vector.tensor_scalar_add(ph[:, :, :], th[:, :, :], PI_2)
                    nc.vector.tensor_scalar_mul(tmp[:, :, :], src[:, :, :], INV_2PI)
                    nc.vector.tensor_copy(tmpi[:, :, :], tmp[:, :, :])
                    nc.vector.tensor_copy(tmp[:, :, :], tmpi[:, :, :])
                    nc.vector.tensor_scalar_mul(tmp[:, :, :], tmp[:, :, :], -TWO_PI)
                    nc.vector.tensor_add(tmp[:, :, :], tmp[:, :, :], src[:, :, :])
                    nc.scalar.activation(dst[:, :, 0, 0:16], tmp[:, :, :], AF.Sin)
                # zero-frequency half: cos=1, sin=0
                nc.gpsimd.tensor_scalar(cos4[:, :, 0, 16:32], th[:, :, :], -1.0, None, op0=mybir.AluOpType.is_ge)
                nc.gpsimd.tensor_scalar(sin4[:, :, 0, 16:32], th[:, :, :], 1e9, None, op0=mybir.AluOpType.is_ge)
                for a in range(1, 4):
                    nc.vector.tensor_copy(cos4[:, :, a, :], cos4[:, :, 0, :])
                    nc.vector.tensor_copy(sin4[:, :, a, :], sin4[:, :, 0, :])

                for g in range(4):
                    for ii in range(4):
                        i = 4 * g + ii
                        ps = QPS.tile([128, 3 * DL], F32, tag="qkvps")
                        for k in range(8):
                            nc.tensor.matmul(ps[:, :], xTa[:, k, 128 * i:128 * (i + 1)],
                                             WTa[:, k, :], start=(k == 0), stop=(k == 7))
                        nc.scalar.copy(qkv[:, i, 0:256], ps[:, 0:256])
                        # v with residual: psum cols 256:320 -> 256:320 ; 320:384 -> 321:385
                        nc.vector.tensor_add(qkv[:, i, 256:320], ps[:, 256:320], ves[:, i, 0:64])
                        nc.vector.tensor_add(qkv[:, i, 321:385], ps[:, 320:384], ves[:, i, 64:128])
                    # ---- norm + rotary for group g (tiles 4g..4g+3) ----
                    sqg = G.tile([128, 4, 256], F32, tag="sqg")
                    for ii in range(4):
                        i = 4 * g + ii
                        nc.scalar.activation(sqg[:, ii, :], qkv[:, i, 0:256].bitcast(F32), AF.Square)
                    # red layout: [128, group4, tile4] so q-groups (0:2) and k-groups (2:4) are contiguous
                    red = G.tile([128, 4, 4], F32, tag="red")
                    nc.vector.tensor_reduce(red[:, :, :].transpose([0, 2, 1]),
                                            sqg[:, :, :].rearrange("p t (a d) -> p t a d", d=64),
                                            axis=mybir.AxisListType.X, op=mybir.AluOpType.add)
                    rno = G.tile([128, 4, 4], F32, tag="rno")
                    nc.scalar.activation(rno[:, 0:2, :], red[:, 0:2, :], AF.Sqrt, scale=scl[:, 0:1])
                    nc.scalar.activation(rno[:, 2:4, :], red[:, 2:4, :], AF.Sqrt, scale=scl[:, 1:2])
                    rin = G.tile([128, 4, 4], F32, tag="rin")
                    nc.vector.reciprocal(rin[:, :, :], rno[:, :, :])
                    for ii in range(4):
                        i = 4 * g + ii
                        for g4 in range(4):
                            nc.vector.tensor_scalar_mul(
                                qkv[:, i, 64 * g4:64 * (g4 + 1)],
                                qkv[:, i, 64 * g4:64 * (g4 + 1)].bitcast(F32),
                                rin[:, g4, ii:ii + 1])
                    # rotary in place: x1 = cols (4g4)*64 .. +32 ; x2 = +32
                    x1 = qkv[:, 4 * g:4 * g + 4, 0:256].rearrange("p t (a d) -> p t a d", d=64)[:, :, :, 0:32]
                    x2 = qkv[:, 4 * g:4 * g + 4, 0:256].rearrange("p t (a d) -> p t a d", d=64)[:, :, :, 32:64]
                    cg = cos4[:, 4 * g:4 * g + 4, :, :]
                    sg = sin4[:, 4 * g:4 * g + 4, :, :]
                    t3 = G.tile([128, 4, 4, 32], F32, tag="t3")
                    t4 = G.tile([128, 4, 4, 32], F32, tag="t4")
                    y2s = G.tile([128, 4, 4, 32], F32, tag="y2s")
                    nc.vector.tensor_mul(t3[:, :, :, :], x1.bitcast(F32), sg)
                    nc.vector.tensor_mul(t4[:, :, :, :], x2.bitcast(F32), cg)
                    nc.vector.tensor_sub(y2s[:, :, :, :], t4[:, :, :, :], t3[:, :, :, :])
                    nc.vector.tensor_mul(t3[:, :, :, :], x1.bitcast(F32), cg)
                    nc.vector.tensor_mul(t4[:, :, :, :], x2.bitcast(F32), sg)
                    nc.vector.tensor_add(x1, t3[:, :, :, :], t4[:, :, :, :])
                    nc.vector.tensor_copy(x2, y2s[:, :, :, :])
                    # ---- transposes of q,k for group ----
                    ptq = TPS.tile([128, 512], F32R, tag="ptq")
                    ptk = TPS.tile([128, 512], F32R, tag="ptk")
                    for ii in range(4):
                        i = 4 * g + ii
                        nc.tensor.transpose(ptq[:, 128 * ii:128 * (ii + 1)], qkv[:, i, 0:128], idn[:, :])
                        nc.tensor.transpose(ptk[:, 128 * ii:128 * (ii + 1)], qkv[:, i, 128:256], idn[:, :])
                    nc.scalar.copy(qrT[:, 512 * g:512 * (g + 1)], ptq[:, :].bitcast(F32))
                    nc.scalar.copy(krT[:, 512 * g:512 * (g + 1)], ptk[:, :].bitcast(F32))

            # ================= attention =================
            with tc.tile_pool(name="sps", bufs=2, space="PSUM") as SPS, \
                 tc.tile_pool(name="yps", bufs=1, space="PSUM") as YPS, \
                 tc.tile_pool(name="eps", bufs=3) as EPS:
                for h in range(2):
                    yw = []
                    for w in range(4):
                        t_ = YPS.tile([65, 512], F32, tag=f"yw{w}")
                        yw.append(t_)
                    for j in range(NT):
                        lk = krT[64 * h:64 * (h + 1), 128 * j:128 * (j + 1)]
                        cs_al = 512 * (j // 4)
                        chunks = [(cs_al, 1024 * (cs_al // 1024 + 1))]
                        q0 = cs_al // 1024 + 1
                        while 1024 * q0 < T:
                            chunks.append((1024 * q0, 1024 * (q0 + 1)))
                            q0 += 1
                        off = 128 * (j % 4)  # diag offset within first chunk
                        for (cs, ce) in chunks:
                            wdt = ce - cs
                            psc = SPS.tile([128, 1024], F32, tag="psc")
                            for p0 in range(cs, ce, 512):
                                nc.tensor.matmul(psc[:, p0 - cs:p0 + 512 - cs], lk,
                                                 qrT[64 * h:64 * (h + 1), p0:p0 + 512],
                                                 start=True, stop=True)
                            es = EPS.tile([128, 1024], F32R, tag="es")
                            nc.scalar.activation(es[:, 0:wdt], psc[:, 0:wdt], AF.Exp)
                            if cs == cs_al:
                                if off > 0:
                                    nc.vector.tensor_scalar_mul(es[:, 0:off], es[:, 0:off].bitcast(F32), 0.0)
                                nc.vector.tensor_mul(es[:, off:off + 128], es[:, off:off + 128].bitcast(F32), msk[:, :])
                            # PV pieces (all full 512, zero-offset)
                            lv = qkv[:, j, 256 + 65 * h:256 + 65 * h + 65]
                            for p0 in range(cs, ce, 512):
                                w = p0 // 512
                                nc.tensor.matmul(yw[w][:, :], lv, es[:, p0 - cs:p0 + 512 - cs],
                                                 start=(j == 0), stop=(j == min(15, 4 * w + 3)))
                    # normalize: recip of denom rows, bcast via ones matmul, divide
                    for w in range(4):
                        c0 = h * T + 512 * w
                        nc.vector.reciprocal(rdf[0:1, c0:c0 + 512], yw[w][64:65, :])
                        nc.vector.tensor_scalar_mul(rd[0:1, c0:c0 + 512], rdf[0:1, c0:c0 + 512], 1.0)
                        pb = SPS.tile([64, 512], F32, tag="psc")
                        nc.tensor.matmul(pb[:, :], on1[:, :], rd[0:1, c0:c0 + 512], start=True, stop=True)
                        ycp = EPS.tile([64, 512], F32, tag="ycp")
                        nc.scalar.copy(ycp[:, :], yw[w][0:64, :])
                        nc.vector.tensor_mul(yT[64 * h:64 * (h + 1), 512 * w:512 * (w + 1)],
                                             ycp[:, :], pb[:, :])

            # ================= output projection + reduce-scatter =================
            with tc.tile_pool(name="ops", bufs=3, space="PSUM") as OPS, \
                 tc.tile_pool(name="ost", bufs=3) as OST:
                for i in range(NT):
                    po = OPS.tile([128, 1024], F32, tag="po")
                    nc.tensor.matmul(po[:, 0:512], yT[:, 128 * i:128 * (i + 1)], WpT[:, 0:512], start=True, stop=True)
                    nc.tensor.matmul(po[:, 512:1024], yT[:, 128 * i:128 * (i + 1)], WpT[:, 512:1024], start=True, stop=True)
                    ob = OST.tile([128, 1024], F32, tag="ob")
                    if i % 2 == 0:
                        nc.scalar.copy(ob[:, :], po[:, :])
                    else:
                        nc.vector.tensor_copy(ob[:, :], po[:, :])
                    nc.sync.dma_start(out=rs_in[128 * i:128 * (i + 1), :], in_=ob[:, :])
                nc.gpsimd.collective_compute(
                    "ReduceScatter", mybir.AluOpType.add, replica_groups=RG,
                    ins=[rs_in[:, :].opt()], outs=[rs_out[:, :].opt()])
                # downcast the final 256 rows to bf16 for the host download
                of = OST.tile([128, 2, DIM], F32, tag="of")
                og = OST.tile([128, 2, DIM], BF16, tag="og")
                for r in range(2):
                    nc.sync.dma_start(out=of[:, r, :], in_=rs_out[128 * r:128 * (r + 1), :])
                nc.vector.tensor_copy(og[:, :, :], of[:, :, :])
                for r in range(2):
                    nc.sync.dma_start(out=d_out[128 * r:128 * (r + 1), :], in_=og[:, r, :])
    nc.compile()
    return nc


def _prep_inputs(x, ve, c_q, c_k, c_v, qkv_scale, q_scale, k_scale, v_lambda, c_proj, c_proj_scale):
    import ml_dtypes
    BF = ml_dtypes.bfloat16
    x = np.asarray(x, np.float32)[0]          # [T, DIM]
    ve = np.asarray(ve, np.float32)[0]
    W = np.asarray(qkv_scale, np.float32)[:, None] * np.concatenate(
        [np.asarray(c_q, np.float32), np.asarray(c_k, np.float32), np.asarray(c_v, np.float32)], axis=0)
    spq = _softplus(float(np.asarray(q_scale)))
    spk = _softplus(float(np.asarray(k_scale)))
    spv = _softplus(float(np.asarray(v_lambda)))

    xT = np.ascontiguousarray(x.T).astype(BF)  # [DIM, T]
    sclf = np.empty((128, 2), np.float32)
    sclf[:, 0] = 1.0 / (spq * spq)
    sclf[:, 1] = 1.0 / (64.0 * spk * spk)
    scl_bf_bytes = sclf.view(BF)  # [128, 4] raw f32 bytes as bf16 pairs

    Wp = np.asarray(c_proj_scale, np.float32)[None, :] * np.asarray(c_proj, np.float32)  # [e, d]
    vesf = spv * ve  # [T, DIM] f32

    in_maps = []
    for c in range(NCORES):
        r0 = DL * c
        Wc = np.concatenate([W[r0:r0 + DL], W[DIM + r0:DIM + r0 + DL], W[2 * DIM + r0:2 * DIM + r0 + DL]], axis=0)  # [384, 1024]
        WTc = np.ascontiguousarray(Wc.T)      # [1024, 384]
        big = np.empty((128, 8196), BF)
        big[:, 0:2048] = xT[128 * c:128 * (c + 1), :]
        big[:, 2048:4096] = vesf[:, r0:r0 + DL].reshape(NT, 128, 128).transpose(1, 0, 2).reshape(128, 2048).astype(BF)
        big[:, 4096:7168] = WTc.reshape(8, 128, 3 * DL).transpose(1, 0, 2).reshape(128, 3072).astype(BF)
        big[:, 7168:8192] = Wp[:, r0:r0 + DL].T.astype(BF)
        big[:, 8192:8196] = scl_bf_bytes
        in_maps.append({"big": big})
    return in_maps


def _enable_jax_compile_cache():
    # content-addressed persistent compile cache: the spmd runner re-traces
    # and re-lowers its jitted body on every invocation, which re-runs the
    # neuronx-cc lowering hook (~0.5s) unless the persistent cache absorbs it
    import os
    try:
        import jax
        jax.config.update("jax_compilation_cache_dir",
                          os.path.expanduser("~/.cache/jax_bass_cc"))
    except Exception:
        pass
    try:
        import jax
        jax.config.update("jax_persistent_cache_min_compile_time_secs", 0.0)
    except Exception:
        pass
    try:
        import jax
        jax.config.update("jax_persistent_cache_min_entry_size_bytes", -1)
    except Exception:
        pass


def kernel(x, ve, c_q, c_k, c_v, qkv_scale, q_scale, k_scale, v_lambda, c_proj, c_proj_scale, _trace=False):
    from concourse.bass_utils import run_bass_kernel_spmd
    _enable_jax_compile_cache()
    if _cache["nc"] is None:
        _cache["nc"] = _build_nc()
    nc = _cache["nc"]
    in_maps = _prep_inputs(x, ve, c_q, c_k, c_v, qkv_scale, q_scale, k_scale, v_lambda, c_proj, c_proj_scale)
    import time as _time
    res = run_bass_kernel_spmd(nc, in_maps, core_ids=list(range(NCORES)))
    res = run_bass_kernel_spmd(nc, in_maps, core_ids=list(range(NCORES)))
    t0 = _time.time()
    res = run_bass_kernel_spmd(nc, in_maps, core_ids=list(range(NCORES)))
    kernel.last_exec_wall_ns = int((_time.time() - t0) * 1e9)
    out = np.concatenate([r["out"] for r in res.results], axis=0)
    kernel.last_results = res
    return np.ascontiguousarray(out.astype(np.float32))[None, :, :]
